# revision 9
# baseline (speedup 1.0000x reference)
"""Causal GQA attention (S=2048, B=2, HQ=32, HKV=8, D=128) on 8 trn2 cores.

Sharding: the 16 (batch, kv-head) pairs are split 2 per core (data+head
parallel). Each pair carries group=4 query heads -> 8 attention heads/core.

Per head the device kernel walks two 1024-wide q-chunks; for each chunk it
streams the causal k-tiles (128 wide): S^T = (Q K^T)^T lands in a 2-bank
PSUM staging tile (k on partitions, q on the free axis), one wide ACTIVATE
exponentiates it into SBUF (P^T, bf16), the 128x128 diagonal block is
masked by a triangular multiply, and V-stationary matmuls accumulate
out^T = V^T P^T into a persistent 2-bank PSUM accumulator. All matmul
operands are bf16 (1 col/cycle on the PE at full clock) and every matmul
is <=512 moving columns so no instruction straddles a PSUM bank.

Softmax denominators: k-tiles with kti%4==0 go straight to the PE as
ones-column matmuls into a shared PSUM sum bank (kti==0 opens the
accumulation with full chunk width); the other k-tiles are element-wise
accumulated on the DVE into a bf16 partial-sum tile, which a single pair
of ones-matmuls folds into the same PSUM rows at chunk end. This keeps
both PE and DVE under the ScalarE exp floor, which is the roofline here
(1 elem/lane/cycle @ 1.2 GHz over ~17.4M causal logits/core).

Chunk tails are software-pipelined: the accumulator is evacuated to SBUF
(bf16) immediately so the next chunk's matmuls can reuse the PSUM bank,
then sums -> SBUF -> DMA-reshape [128,8] -> reciprocal -> DRAM ->
partition-broadcast -> multiply -> store advances one stage per k-tile
iteration, several chunks in flight.

Host side only re-lays-out data (and casts to bf16): Q/K as [d, s], V as
[k_local, ktile*d]; the returned out^T [d, s] is transposed/cast back.
"""

import numpy as np
import ml_dtypes

import concourse.bass as bass
import concourse.mybir as mybir
import concourse.tile as tile
from concourse import bacc, bass_utils
from concourse.masks import make_upper_triangular

S, B, HQ, HKV, D = 2048, 2, 32, 8, 128
G = HQ // HKV                      # 4 query heads per kv head
NCORES = 8
NPAIRS = B * HKV                   # 16 (batch, kv-head) pairs
PAIRS_PER_CORE = NPAIRS // NCORES  # 2
HEADS_PER_CORE = PAIRS_PER_CORE * G  # 8
SCALE = 1.0 / float(np.sqrt(D))
CH = 1024                          # q-chunk width (2 PSUM banks)
NCH = S // CH                      # 2
KT = 128                           # k-tile (partition) width
NKT = S // KT                      # 16

F32 = mybir.dt.float32
BF16 = mybir.dt.bfloat16
NP_BF16 = ml_dtypes.bfloat16


def _segs(off):
    """Split chunk cols [off, CH) into <=512 pieces that don't straddle
    the 512 boundary (one PSUM bank per matmul)."""
    if off < 512:
        return [(off, 512), (512, CH)]
    return [(off, CH)]


def _sum_rc(row_base, s0, s1):
    """Map chunk cols [s0, s1) to (row, col range) in the 512-wide sum
    bank: lo half at row_base, hi half at row_base+32."""
    if s0 < 512:
        return row_base, s0, s1
    return row_base + 32, s0 - 512, s1 - 512


def emit_core_program(tc, qt, kt, v, recd, ot):
    from contextlib import ExitStack

    nc = tc.nc
    with ExitStack() as ctx:
        _emit_core_program(ctx, tc, nc, qt, kt, v, recd, ot)


def _emit_core_program(ctx, tc, nc, qt, kt, v, recd, ot):
    singles = ctx.enter_context(tc.tile_pool(name="singles", bufs=1))
    kv_pool = ctx.enter_context(tc.tile_pool(name="kv", bufs=2))
    q_pool = ctx.enter_context(tc.tile_pool(name="q", bufs=2))
    pt_pool = ctx.enter_context(tc.tile_pool(name="pt", bufs=5))
    sacc_pool = ctx.enter_context(tc.tile_pool(name="sacc", bufs=2))
    osb_pool = ctx.enter_context(tc.tile_pool(name="osb", bufs=3))
    bcs_pool = ctx.enter_context(tc.tile_pool(name="bcs", bufs=3))
    srow_pool = ctx.enter_context(tc.tile_pool(name="srow", bufs=3))
    srec_pool = ctx.enter_context(tc.tile_pool(name="srec", bufs=3))
    st_pool = ctx.enter_context(tc.tile_pool(name="st", bufs=2, space="PSUM"))
    oa_pool = ctx.enter_context(tc.tile_pool(name="oa", bufs=1, space="PSUM"))
    ps_sum = ctx.enter_context(tc.tile_pool(name="ps_sum", bufs=1, space="PSUM"))

    # Constants: tri[k, q] = 1.0 where q >= k (allowed), 0.0 where q < k.
    trif = singles.tile([128, 128], F32)
    make_upper_triangular(nc, trif[:], val=1.0, diag=True)
    tri = singles.tile([128, 128], BF16)
    nc.scalar.copy(out=tri[:], in_=trif[:])
    onesc = singles.tile([128, 1], BF16)   # ones column (sum-over-k lhsT)
    nc.vector.memset(onesc[:], 1.0)

    # One sum bank for the whole program; rows 0/32 and 64/96 alternate by
    # global chunk parity (subtile deps keep the parities independent).
    sum_ps = ps_sum.tile([128, 512], F32)

    kv_tiles = {}

    def ensure_pair(pair):
        if pair in kv_tiles or pair >= PAIRS_PER_CORE:
            return
        kt_sb = kv_pool.tile([D, S], BF16, tag="kt", name=f"kt_{pair}")
        nc.sync.dma_start(out=kt_sb[:], in_=kt[pair])
        v_sb = kv_pool.tile([128, NKT * D], BF16, tag="v", name=f"v_{pair}")
        nc.sync.dma_start(out=v_sb[:], in_=v[pair])
        kv_tiles[pair] = (kt_sb, v_sb)

    q_tiles = {}

    def ensure_head(head):
        if head in q_tiles or head >= HEADS_PER_CORE:
            return
        q_sb = q_pool.tile([D, S], BF16, tag="q", name=f"q_{head}")
        nc.sync.dma_start(out=q_sb[:], in_=qt[head])
        q_tiles[head] = q_sb

    # Flat schedule: (head, chunk, kti)
    sched = []
    for head in range(HEADS_PER_CORE):
        for c in range(NCH):
            for kti in range(8 * c + 8):
                sched.append((head, c, kti))

    # Per-(head,chunk) live state filled in while emitting
    oacc = {}      # (head, c) -> psum accumulator tile
    saccs = {}     # (head, c) -> (tile, base_off)
    stages = {}    # sched index -> staging tile

    # Chunk-tail normalization pipeline, advanced one stage per iteration
    pending = []

    def advance_norm(drain=False):
        for ent in list(pending):
            head, c, st = ent["head"], ent["c"], ent["stage"]
            if st == 0:
                srec = srec_pool.tile([128, NCH * 4], F32, tag="srec",
                                      name=f"srec_{head}_{c}")
                nc.sync.dma_start(out=srec[:], in_=ent["srow"][:])
                ent["srec"] = srec
            elif st == 1:
                srec2 = srec_pool.tile([128, NCH * 4], BF16, tag="srec2",
                                       name=f"srec2_{head}_{c}")
                with nc.allow_low_precision(reason="1/sum broadcast in bf16"):
                    nc.vector.reciprocal(out=srec2[:], in_=ent["srec"][:])
                nc.sync.dma_start(out=recd[head, c], in_=srec2[:])
            elif st == 2:
                bcs = bcs_pool.tile([128, CH], BF16, tag="bcs", name=f"bcs_{head}_{c}")
                nc.sync.dma_start(
                    out=bcs[:], in_=recd[head, c].partition_broadcast(128))
                ent["bcs"] = bcs
            elif st == 3:
                osb2 = osb_pool.tile([128, CH], BF16, tag="osb2",
                                     name=f"osb2_{head}_{c}")
                nc.vector.tensor_mul(osb2[:], ent["osb"][:], ent["bcs"][:])
                nc.sync.dma_start(
                    out=ot[head][:, CH * c:CH * (c + 1)], in_=osb2[:])
                pending.remove(ent)
            ent["stage"] = st + 1

    def emit_qk(i):
        head, c, kti = sched[i]
        if c == 0 and kti == 0:
            pair = head // G
            ensure_pair(pair + 1)
            ensure_head(head + 1)
        kt_sb, _ = kv_tiles[head // G]
        q_sb = q_tiles[head]
        off = max(0, 128 * kti - CH * c)
        stage = st_pool.tile([128, CH], F32, tag="stage", name=f"st_{i}")
        for (s0, s1) in _segs(off):
            nc.tensor.matmul(
                out=stage[:, s0:s1],
                lhsT=kt_sb[:, 128 * kti:128 * (kti + 1)],
                rhs=q_sb[:, CH * c + s0:CH * c + s1],
                start=True, stop=True,
            )
        stages[i] = stage

    def emit_rest(i):
        head, c, kti = sched[i]
        _, v_sb = kv_tiles[head // G]
        off = max(0, 128 * kti - CH * c)
        last = kti == 8 * c + 7
        row_base = 64 * ((head * NCH + c) % 2)
        stage = stages.pop(i)

        # exp into SBUF (bf16); one wide ACTIVATE per k-tile
        p_kt = pt_pool.tile([128, CH], BF16, tag="pt", name=f"pt_{i}")
        nc.scalar.activation(
            p_kt[:, off:CH], stage[:, off:CH],
            mybir.ActivationFunctionType.Exp, scale=SCALE)

        # causal mask on the diagonal 128x128 block
        if 128 * kti >= CH * c:
            nc.vector.tensor_mul(
                p_kt[:, off:off + 128], p_kt[:, off:off + 128], tri[:])

        # out^T += V^T P^T
        if kti == 0:
            oacc[(head, c)] = oa_pool.tile(
                [128, CH], F32, tag="oacc", name=f"oa_{head}_{c}")
        oa = oacc[(head, c)]
        for (s0, s1) in _segs(off):
            nc.tensor.matmul(
                out=oa[:, s0:s1],
                lhsT=v_sb[:, D * kti:D * (kti + 1)],
                rhs=p_kt[:, s0:s1],
                start=(kti == 0), stop=last,
            )

        # denominators: kti%4==0 -> PE ones-matmul, else GpSimd accumulate
        # (GpSimd is otherwise idle and this chain is only needed at chunk
        # end, so it stays off the exp->AV critical path)
        if kti % 4 == 0:
            for (s0, s1) in _segs(off):
                r, c0, c1 = _sum_rc(row_base, s0, s1)
                nc.tensor.matmul(
                    out=sum_ps[r:r + 1, c0:c1],
                    lhsT=onesc[:],
                    rhs=p_kt[:, s0:s1],
                    start=(kti == 0), stop=False,
                    tile_position=(0, r),
                )
        else:
            key = (head, c)
            if key not in saccs:
                sacc = sacc_pool.tile([128, CH], BF16, tag="sacc", name=f"sacc_{head}_{c}")
                nc.gpsimd.tensor_copy(sacc[:, off:CH], p_kt[:, off:CH])
                saccs[key] = (sacc, off)
            else:
                sacc, _ = saccs[key]
                nc.gpsimd.tensor_add(
                    sacc[:, off:CH], sacc[:, off:CH], p_kt[:, off:CH])

        if last:
            # fold the DVE partial sums into the PSUM sum rows
            sacc, base = saccs.pop((head, c))
            for (s0, s1) in _segs(base):
                r, c0, c1 = _sum_rc(row_base, s0, s1)
                nc.tensor.matmul(
                    out=sum_ps[r:r + 1, c0:c1],
                    lhsT=onesc[:],
                    rhs=sacc[:, s0:s1],
                    start=False, stop=True,
                    tile_position=(0, r),
                )
            # evacuate the accumulator so the next chunk can reuse the bank
            oa = oacc.pop((head, c))
            osb = osb_pool.tile([128, CH], BF16, tag="osb",
                                name=f"osb_{head}_{c}")
            nc.vector.tensor_copy(osb[:], oa[:])
            # pull the two sum rows out of PSUM (single partition, 1024 wide)
            srow = srow_pool.tile([1, CH], F32, tag="srow", name=f"srow_{head}_{c}")
            nc.vector.tensor_copy(srow[0:1, 0:512], sum_ps[row_base:row_base + 1, :])
            nc.vector.tensor_copy(
                srow[0:1, 512:CH], sum_ps[row_base + 32:row_base + 33, :])
            pending.append(dict(head=head, c=c, stage=0, srow=srow, osb=osb))

    ensure_pair(0)
    ensure_head(0)
    emit_qk(0)
    for i in range(len(sched)):
        if i + 1 < len(sched):
            emit_qk(i + 1)
        emit_rest(i)
        advance_norm()
    while pending:
        advance_norm(drain=True)


_CACHED_NC = None


def build_program():
    global _CACHED_NC
    if _CACHED_NC is not None:
        return _CACHED_NC
    nc = bacc.Bacc("TRN2", target_bir_lowering=False, debug=False,
                   num_devices=NCORES)
    qt = nc.dram_tensor("qt", [HEADS_PER_CORE, D, S], BF16,
                        kind="ExternalInput").ap()
    kt = nc.dram_tensor("kt", [PAIRS_PER_CORE, D, S], BF16,
                        kind="ExternalInput").ap()
    v = nc.dram_tensor("v", [PAIRS_PER_CORE, 128, NKT * D], BF16,
                       kind="ExternalInput").ap()
    recd = nc.dram_tensor("recd", [HEADS_PER_CORE, NCH, CH], BF16,
                          kind="Internal").ap()
    ot = nc.dram_tensor("ot", [HEADS_PER_CORE, D, S], BF16,
                        kind="ExternalOutput").ap()
    with tile.TileContext(nc) as tc:
        emit_core_program(tc, qt, kt, v, recd, ot)
    nc.compile()
    _CACHED_NC = nc
    return nc


def shard_inputs(query, key, value):
    """Full inputs -> list of 8 per-core in_maps (host relayout + bf16)."""
    query = np.asarray(query, dtype=np.float32)
    key = np.asarray(key, dtype=np.float32)
    value = np.asarray(value, dtype=np.float32)

    # Q: [S,B,HQ,D] -> [B*HKV, G, D, S]
    qtall = np.ascontiguousarray(
        query.reshape(S, B, HKV, G, D).transpose(1, 2, 3, 4, 0)
    ).reshape(NPAIRS, G, D, S).astype(NP_BF16)
    # K: [S,B,HKV,D] -> [B*HKV, D, S]
    ktall = np.ascontiguousarray(
        key.transpose(1, 2, 3, 0)).reshape(NPAIRS, D, S).astype(NP_BF16)
    # V: [S,B,HKV,D] -> [B*HKV, k_local=128, NKT*D]
    vall = np.ascontiguousarray(
        value.reshape(NKT, 128, B, HKV, D).transpose(2, 3, 1, 0, 4)
    ).reshape(NPAIRS, 128, NKT * D).astype(NP_BF16)

    in_maps = []
    for c in range(NCORES):
        p0 = PAIRS_PER_CORE * c
        p1 = p0 + PAIRS_PER_CORE
        in_maps.append({
            "qt": np.ascontiguousarray(qtall[p0:p1].reshape(HEADS_PER_CORE, D, S)),
            "kt": np.ascontiguousarray(ktall[p0:p1]),
            "v": np.ascontiguousarray(vall[p0:p1]),
        })
    return in_maps


def unshard_output(results):
    """8 per-core {'ot': [8, D, S]} -> full [S, B, HQ, D]."""
    ot = np.stack([np.asarray(r["ot"], dtype=np.float32) for r in results])
    ot = ot.reshape(B, HKV, G, D, S)                   # pairs major -> b, hkv
    out = np.ascontiguousarray(ot.transpose(4, 0, 1, 2, 3))  # [S,B,HKV,G,D]
    return out.reshape(S, B, HQ, D)


def kernel(query, key, value, _trace=False, _return_bkr=False):
    nc = build_program()
    in_maps = shard_inputs(query, key, value)
    bkr = bass_utils.run_bass_kernel_spmd(
        nc, in_maps, core_ids=list(range(NCORES)), trace=_trace)
    out = unshard_output(bkr.results)
    if _return_bkr:
        return out, bkr
    return out


if __name__ == "__main__":
    q = np.random.randn(S, B, HQ, D).astype(np.float32)
    k = np.random.randn(S, B, HKV, D).astype(np.float32)
    vv = np.random.randn(S, B, HKV, D).astype(np.float32)
    o = kernel(q, k, vv)
    print("out", o.shape, o.dtype, float(np.abs(o).max()))


# revision 12
# speedup vs baseline: 1.3126x; 1.3126x over previous
"""Causal GQA attention (S=2048, B=2, HQ=32, HKV=8, D=128) on 8 trn2 cores.

Sharding: the 16 (batch, kv-head) pairs are split 2 per core (data+head
parallel). Each pair carries group=4 query heads -> 8 attention heads/core.

Per head the device kernel walks two 1024-wide q-chunks; for each chunk it
streams the causal k-tiles (128 wide): S^T = (Q K^T)^T lands in a 2-bank
PSUM staging tile (k on partitions, q on the free axis), one wide ACTIVATE
exponentiates it into SBUF (P^T, bf16), the 128x128 diagonal block is
masked by a triangular multiply, and V-stationary matmuls accumulate
out^T = V^T P^T into a persistent 2-bank PSUM accumulator. All matmul
operands are bf16 (1 col/cycle on the PE at full clock) and every matmul
is <=512 moving columns so no instruction straddles a PSUM bank.

Softmax denominators: k-tiles with kti%4==0 go straight to the PE as
ones-column matmuls into a shared PSUM sum bank (kti==0 opens the
accumulation with full chunk width); the other k-tiles are element-wise
accumulated on the DVE into a bf16 partial-sum tile, which a single pair
of ones-matmuls folds into the same PSUM rows at chunk end. This keeps
both PE and DVE under the ScalarE exp floor, which is the roofline here
(1 elem/lane/cycle @ 1.2 GHz over ~17.4M causal logits/core).

Chunk tails are software-pipelined: the accumulator is evacuated to SBUF
(bf16) immediately so the next chunk's matmuls can reuse the PSUM bank,
then sums -> SBUF -> DMA-reshape [128,8] -> reciprocal -> DRAM ->
partition-broadcast -> multiply -> store advances one stage per k-tile
iteration, several chunks in flight.

Host side only re-lays-out data (and casts to bf16): Q/K as [d, s], V as
[k_local, ktile*d]; the returned out^T [d, s] is transposed/cast back.
"""

import numpy as np
import ml_dtypes

import concourse.bass as bass
import concourse.mybir as mybir
import concourse.tile as tile
from concourse import bacc, bass_utils
from concourse.masks import make_upper_triangular

S, B, HQ, HKV, D = 2048, 2, 32, 8, 128
G = HQ // HKV                      # 4 query heads per kv head
NCORES = 8
NPAIRS = B * HKV                   # 16 (batch, kv-head) pairs
PAIRS_PER_CORE = NPAIRS // NCORES  # 2
HEADS_PER_CORE = PAIRS_PER_CORE * G  # 8
SCALE = 1.0 / float(np.sqrt(D))
CH = 1024                          # q-chunk width (2 PSUM banks)
NCH = S // CH                      # 2
KT = 128                           # k-tile (partition) width
NKT = S // KT                      # 16

F32 = mybir.dt.float32
BF16 = mybir.dt.bfloat16
NP_BF16 = ml_dtypes.bfloat16


def _segs(off):
    """Split chunk cols [off, CH) into <=512 pieces that don't straddle
    the 512 boundary (one PSUM bank per matmul)."""
    if off < 512:
        return [(off, 512), (512, CH)]
    return [(off, CH)]


def _sum_rc(row_base, s0, s1):
    """Map chunk cols [s0, s1) to (row, col range) in the 512-wide sum
    bank: lo half at row_base, hi half at row_base+32."""
    if s0 < 512:
        return row_base, s0, s1
    return row_base + 32, s0 - 512, s1 - 512


def emit_core_program(tc, qt, kt, v, recd, ot):
    from contextlib import ExitStack

    nc = tc.nc
    with ExitStack() as ctx:
        _emit_core_program(ctx, tc, nc, qt, kt, v, recd, ot)


def _emit_core_program(ctx, tc, nc, qt, kt, v, recd, ot):
    singles = ctx.enter_context(tc.tile_pool(name="singles", bufs=1))
    kv_pool = ctx.enter_context(tc.tile_pool(name="kv", bufs=2))
    q_pool = ctx.enter_context(tc.tile_pool(name="q", bufs=2))
    pt_pool = ctx.enter_context(tc.tile_pool(name="pt", bufs=5))
    sacc_pool = ctx.enter_context(tc.tile_pool(name="sacc", bufs=2))
    osb_pool = ctx.enter_context(tc.tile_pool(name="osb", bufs=3))
    bcs_pool = ctx.enter_context(tc.tile_pool(name="bcs", bufs=3))
    srow_pool = ctx.enter_context(tc.tile_pool(name="srow", bufs=3))
    srec_pool = ctx.enter_context(tc.tile_pool(name="srec", bufs=3))
    st_pool = ctx.enter_context(tc.tile_pool(name="st", bufs=2, space="PSUM"))
    oa_pool = ctx.enter_context(tc.tile_pool(name="oa", bufs=1, space="PSUM"))
    ps_sum = ctx.enter_context(tc.tile_pool(name="ps_sum", bufs=1, space="PSUM"))

    # Constants: tri[k, q] = 1.0 where q >= k (allowed), 0.0 where q < k.
    trif = singles.tile([128, 128], F32)
    make_upper_triangular(nc, trif[:], val=1.0, diag=True)
    tri = singles.tile([128, 128], BF16)
    nc.scalar.copy(out=tri[:], in_=trif[:])
    onesc = singles.tile([128, 1], BF16)   # ones column (sum-over-k lhsT)
    nc.vector.memset(onesc[:], 1.0)

    # One sum bank for the whole program; rows 0/32 and 64/96 alternate by
    # global chunk parity (subtile deps keep the parities independent).
    sum_ps = ps_sum.tile([128, 512], F32)

    kv_tiles = {}

    def ensure_pair(pair):
        if pair in kv_tiles or pair >= PAIRS_PER_CORE:
            return
        kt_sb = kv_pool.tile([D, S], BF16, tag="kt", name=f"kt_{pair}")
        nc.sync.dma_start(out=kt_sb[:], in_=kt[pair])
        v_sb = kv_pool.tile([128, NKT * D], BF16, tag="v", name=f"v_{pair}")
        nc.sync.dma_start(out=v_sb[:], in_=v[pair])
        kv_tiles[pair] = (kt_sb, v_sb)

    q_tiles = {}

    def ensure_head(head):
        if head in q_tiles or head >= HEADS_PER_CORE:
            return
        q_sb = q_pool.tile([D, S], BF16, tag="q", name=f"q_{head}")
        nc.sync.dma_start(out=q_sb[:], in_=qt[head])
        q_tiles[head] = q_sb

    # Flat schedule: (head, chunk, kti)
    sched = []
    for head in range(HEADS_PER_CORE):
        for c in range(NCH):
            for kti in range(8 * c + 8):
                sched.append((head, c, kti))

    # Per-(head,chunk) live state filled in while emitting
    oacc = {}      # (head, c) -> psum accumulator tile
    saccs = {}     # (head, c) -> (tile, base_off)
    stages = {}    # sched index -> staging tile

    # Chunk-tail normalization pipeline, advanced one stage per iteration
    pending = []

    def advance_norm(drain=False):
        for ent in list(pending):
            head, c, st = ent["head"], ent["c"], ent["stage"]
            if st == 0:
                # pull the two sum rows out of PSUM (single partition,
                # 1024 wide) and DMA-reshape them to [128, 8]
                rb = ent["row_base"]
                srow = srow_pool.tile([1, CH], F32, tag="srow",
                                      name=f"srow_{head}_{c}")
                nc.vector.tensor_copy(srow[0:1, 0:512], sum_ps[rb:rb + 1, :])
                nc.vector.tensor_copy(
                    srow[0:1, 512:CH], sum_ps[rb + 32:rb + 33, :])
                srec = srec_pool.tile([128, NCH * 4], F32, tag="srec",
                                      name=f"srec_{head}_{c}")
                nc.sync.dma_start(out=srec[:], in_=srow[:])
                ent["srec"] = srec
            elif st == 1:
                srec2 = srec_pool.tile([128, NCH * 4], BF16, tag="srec2",
                                       name=f"srec2_{head}_{c}")
                with nc.allow_low_precision(reason="1/sum broadcast in bf16"):
                    nc.vector.reciprocal(out=srec2[:], in_=ent["srec"][:])
                nc.sync.dma_start(out=recd[head, c], in_=srec2[:])
            elif st == 2:
                bcs = bcs_pool.tile([128, CH], BF16, tag="bcs", name=f"bcs_{head}_{c}")
                nc.sync.dma_start(
                    out=bcs[:], in_=recd[head, c].partition_broadcast(128))
                ent["bcs"] = bcs
            elif st == 3:
                osb2 = osb_pool.tile([128, CH], BF16, tag="osb2",
                                     name=f"osb2_{head}_{c}")
                nc.vector.tensor_mul(osb2[:], ent["osb"][:], ent["bcs"][:])
                nc.sync.dma_start(
                    out=ot[head][:, CH * c:CH * (c + 1)], in_=osb2[:])
                pending.remove(ent)
            ent["stage"] = st + 1

    def emit_qk(i):
        head, c, kti = sched[i]
        if c == 0 and kti == 0:
            pair = head // G
            ensure_pair(pair + 1)
            ensure_head(head + 1)
        kt_sb, _ = kv_tiles[head // G]
        q_sb = q_tiles[head]
        off = max(0, 128 * kti - CH * c)
        stage = st_pool.tile([128, CH], F32, tag="stage", name=f"st_{i}")
        for (s0, s1) in _segs(off):
            nc.tensor.matmul(
                out=stage[:, s0:s1],
                lhsT=kt_sb[:, 128 * kti:128 * (kti + 1)],
                rhs=q_sb[:, CH * c + s0:CH * c + s1],
                start=True, stop=True,
            )
        stages[i] = stage

    def emit_rest(i):
        head, c, kti = sched[i]
        _, v_sb = kv_tiles[head // G]
        off = max(0, 128 * kti - CH * c)
        last = kti == 8 * c + 7
        row_base = 64 * ((head * NCH + c) % 2)
        stage = stages.pop(i)

        # exp into SBUF (bf16); one wide ACTIVATE per k-tile
        p_kt = pt_pool.tile([128, CH], BF16, tag="pt", name=f"pt_{i}")
        nc.scalar.activation(
            p_kt[:, off:CH], stage[:, off:CH],
            mybir.ActivationFunctionType.Exp, scale=SCALE)

        # causal mask on the diagonal 128x128 block
        if 128 * kti >= CH * c:
            nc.vector.tensor_mul(
                p_kt[:, off:off + 128], p_kt[:, off:off + 128], tri[:])

        # out^T += V^T P^T
        if kti == 0:
            oacc[(head, c)] = oa_pool.tile(
                [128, CH], F32, tag="oacc", name=f"oa_{head}_{c}")
        oa = oacc[(head, c)]
        for (s0, s1) in _segs(off):
            nc.tensor.matmul(
                out=oa[:, s0:s1],
                lhsT=v_sb[:, D * kti:D * (kti + 1)],
                rhs=p_kt[:, s0:s1],
                start=(kti == 0), stop=last,
            )

        # denominators: a PE/DVE split tuned so both engines land just
        # under the ScalarE exp floor (~169us/core): k-tiles {0,2,4,8,12}
        # go to the PE as ones-column matmuls, the rest accumulate
        # element-wise on the DVE
        if kti % 4 == 0 or kti == 2:
            for (s0, s1) in _segs(off):
                r, c0, c1 = _sum_rc(row_base, s0, s1)
                nc.tensor.matmul(
                    out=sum_ps[r:r + 1, c0:c1],
                    lhsT=onesc[:],
                    rhs=p_kt[:, s0:s1],
                    start=(kti == 0), stop=False,
                    tile_position=(0, r),
                )
        else:
            key = (head, c)
            if key not in saccs:
                sacc = sacc_pool.tile([128, CH], BF16, tag="sacc", name=f"sacc_{head}_{c}")
                nc.vector.tensor_copy(sacc[:, off:CH], p_kt[:, off:CH])
                saccs[key] = (sacc, off)
            else:
                sacc, _ = saccs[key]
                nc.vector.tensor_add(
                    sacc[:, off:CH], sacc[:, off:CH], p_kt[:, off:CH])

        if last:
            # fold the DVE partial sums into the PSUM sum rows
            sacc, base = saccs.pop((head, c))
            for (s0, s1) in _segs(base):
                r, c0, c1 = _sum_rc(row_base, s0, s1)
                nc.tensor.matmul(
                    out=sum_ps[r:r + 1, c0:c1],
                    lhsT=onesc[:],
                    rhs=sacc[:, s0:s1],
                    start=False, stop=True,
                    tile_position=(0, r),
                )
            # evacuate the accumulator so the next chunk can reuse the bank
            oa = oacc.pop((head, c))
            osb = osb_pool.tile([128, CH], BF16, tag="osb",
                                name=f"osb_{head}_{c}")
            nc.vector.tensor_copy(osb[:], oa[:])
            pending.append(dict(head=head, c=c, stage=0, osb=osb,
                                row_base=row_base))

    ensure_pair(0)
    ensure_head(0)
    emit_qk(0)
    for i in range(len(sched)):
        if i + 1 < len(sched):
            emit_qk(i + 1)
        emit_rest(i)
        advance_norm()
    while pending:
        advance_norm(drain=True)


_CACHED_NC = None


def build_program():
    global _CACHED_NC
    if _CACHED_NC is not None:
        return _CACHED_NC
    nc = bacc.Bacc("TRN2", target_bir_lowering=False, debug=False,
                   num_devices=NCORES)
    qt = nc.dram_tensor("qt", [HEADS_PER_CORE, D, S], BF16,
                        kind="ExternalInput").ap()
    kt = nc.dram_tensor("kt", [PAIRS_PER_CORE, D, S], BF16,
                        kind="ExternalInput").ap()
    v = nc.dram_tensor("v", [PAIRS_PER_CORE, 128, NKT * D], BF16,
                       kind="ExternalInput").ap()
    recd = nc.dram_tensor("recd", [HEADS_PER_CORE, NCH, CH], BF16,
                          kind="Internal").ap()
    ot = nc.dram_tensor("ot", [HEADS_PER_CORE, D, S], BF16,
                        kind="ExternalOutput").ap()
    with tile.TileContext(nc) as tc:
        emit_core_program(tc, qt, kt, v, recd, ot)
    nc.compile()
    _CACHED_NC = nc
    return nc


def shard_inputs(query, key, value):
    """Full inputs -> list of 8 per-core in_maps (host relayout + bf16)."""
    query = np.asarray(query, dtype=np.float32)
    key = np.asarray(key, dtype=np.float32)
    value = np.asarray(value, dtype=np.float32)

    # Q: [S,B,HQ,D] -> [B*HKV, G, D, S]
    qtall = np.ascontiguousarray(
        query.reshape(S, B, HKV, G, D).transpose(1, 2, 3, 4, 0)
    ).reshape(NPAIRS, G, D, S).astype(NP_BF16)
    # K: [S,B,HKV,D] -> [B*HKV, D, S]
    ktall = np.ascontiguousarray(
        key.transpose(1, 2, 3, 0)).reshape(NPAIRS, D, S).astype(NP_BF16)
    # V: [S,B,HKV,D] -> [B*HKV, k_local=128, NKT*D]
    vall = np.ascontiguousarray(
        value.reshape(NKT, 128, B, HKV, D).transpose(2, 3, 1, 0, 4)
    ).reshape(NPAIRS, 128, NKT * D).astype(NP_BF16)

    in_maps = []
    for c in range(NCORES):
        p0 = PAIRS_PER_CORE * c
        p1 = p0 + PAIRS_PER_CORE
        in_maps.append({
            "qt": np.ascontiguousarray(qtall[p0:p1].reshape(HEADS_PER_CORE, D, S)),
            "kt": np.ascontiguousarray(ktall[p0:p1]),
            "v": np.ascontiguousarray(vall[p0:p1]),
        })
    return in_maps


def unshard_output(results):
    """8 per-core {'ot': [8, D, S]} -> full [S, B, HQ, D]."""
    ot = np.stack([np.asarray(r["ot"], dtype=np.float32) for r in results])
    ot = ot.reshape(B, HKV, G, D, S)                   # pairs major -> b, hkv
    out = np.ascontiguousarray(ot.transpose(4, 0, 1, 2, 3))  # [S,B,HKV,G,D]
    return out.reshape(S, B, HQ, D)


def kernel(query, key, value, _trace=False, _return_bkr=False):
    nc = build_program()
    in_maps = shard_inputs(query, key, value)
    bkr = bass_utils.run_bass_kernel_spmd(
        nc, in_maps, core_ids=list(range(NCORES)), trace=_trace)
    out = unshard_output(bkr.results)
    if _return_bkr:
        return out, bkr
    return out


if __name__ == "__main__":
    q = np.random.randn(S, B, HQ, D).astype(np.float32)
    k = np.random.randn(S, B, HKV, D).astype(np.float32)
    vv = np.random.randn(S, B, HKV, D).astype(np.float32)
    o = kernel(q, k, vv)
    print("out", o.shape, o.dtype, float(np.abs(o).max()))


# revision 20
# speedup vs baseline: 1.3396x; 1.0206x over previous
"""Causal GQA attention (S=2048, B=2, HQ=32, HKV=8, D=128) on 8 trn2 cores.

Sharding: the 16 (batch, kv-head) pairs are split 2 per core (data+head
parallel). Each pair carries group=4 query heads -> 8 attention heads/core.

Per head the device kernel walks two 1024-wide q-chunks; for each chunk it
streams the causal k-tiles (128 wide): S^T = (Q K^T)^T lands in a 2-bank
PSUM staging tile (k on partitions, q on the free axis), one wide ACTIVATE
exponentiates it into SBUF (P^T, bf16), the 128x128 diagonal block is
masked by a triangular multiply, and V-stationary matmuls accumulate
out^T = V^T P^T into a persistent 2-bank PSUM accumulator. All matmul
operands are bf16 (1 col/cycle on the PE at full clock) and every matmul
is <=512 moving columns so no instruction straddles a PSUM bank.

Softmax denominators: k-tiles with kti%4==0 go straight to the PE as
ones-column matmuls into a shared PSUM sum bank (kti==0 opens the
accumulation with full chunk width); the other k-tiles are element-wise
accumulated on the DVE into a bf16 partial-sum tile, which a single pair
of ones-matmuls folds into the same PSUM rows at chunk end. This keeps
both PE and DVE under the ScalarE exp floor, which is the roofline here
(1 elem/lane/cycle @ 1.2 GHz over ~17.4M causal logits/core).

Chunk tails are software-pipelined: the accumulator is evacuated to SBUF
(bf16) immediately so the next chunk's matmuls can reuse the PSUM bank,
then sums -> SBUF -> DMA-reshape [128,8] -> reciprocal -> DRAM ->
partition-broadcast -> multiply -> store advances one stage per k-tile
iteration, several chunks in flight.

Host side only re-lays-out data (and casts to bf16): Q/K as [d, s], V as
[k_local, ktile*d]; the returned out^T [d, s] is transposed/cast back.
"""

import numpy as np
import ml_dtypes

import concourse.bass as bass
import concourse.mybir as mybir
import concourse.tile as tile
from concourse import bacc, bass_utils
from concourse.masks import make_upper_triangular

S, B, HQ, HKV, D = 2048, 2, 32, 8, 128
G = HQ // HKV                      # 4 query heads per kv head
NCORES = 8
NPAIRS = B * HKV                   # 16 (batch, kv-head) pairs
PAIRS_PER_CORE = NPAIRS // NCORES  # 2
HEADS_PER_CORE = PAIRS_PER_CORE * G  # 8
SCALE = 1.0 / float(np.sqrt(D))
CH = 1024                          # q-chunk width (2 PSUM banks)
NCH = S // CH                      # 2
KT = 128                           # k-tile (partition) width
NKT = S // KT                      # 16

F32 = mybir.dt.float32
BF16 = mybir.dt.bfloat16
NP_BF16 = ml_dtypes.bfloat16


def _segs(off):
    """Split chunk cols [off, CH) into <=512 pieces that don't straddle
    the 512 boundary (one PSUM bank per matmul)."""
    if off < 512:
        return [(off, 512), (512, CH)]
    return [(off, CH)]


def _sum_rc(row_base, s0, s1):
    """Map chunk cols [s0, s1) to (row, col range) in the 512-wide sum
    bank: lo half at row_base, hi half at row_base+32."""
    if s0 < 512:
        return row_base, s0, s1
    return row_base + 32, s0 - 512, s1 - 512


def emit_core_program(tc, qt, kt, v, recd, ot):
    from contextlib import ExitStack

    nc = tc.nc
    with ExitStack() as ctx:
        _emit_core_program(ctx, tc, nc, qt, kt, v, recd, ot)


def _emit_core_program(ctx, tc, nc, qt, kt, v, recd, ot):
    singles = ctx.enter_context(tc.tile_pool(name="singles", bufs=1))
    kv_pool = ctx.enter_context(tc.tile_pool(name="kv", bufs=2))
    q_pool = ctx.enter_context(tc.tile_pool(name="q", bufs=2))
    pt_pool = ctx.enter_context(tc.tile_pool(name="pt", bufs=5))
    sacc_pool = ctx.enter_context(tc.tile_pool(name="sacc", bufs=2))
    osb_pool = ctx.enter_context(tc.tile_pool(name="osb", bufs=3))
    bcs_pool = ctx.enter_context(tc.tile_pool(name="bcs", bufs=3))
    srow_pool = ctx.enter_context(tc.tile_pool(name="srow", bufs=3))
    srec_pool = ctx.enter_context(tc.tile_pool(name="srec", bufs=3))
    st_pool = ctx.enter_context(tc.tile_pool(name="st", bufs=2, space="PSUM"))
    oa_pool = ctx.enter_context(tc.tile_pool(name="oa", bufs=1, space="PSUM"))
    ps_sum = ctx.enter_context(tc.tile_pool(name="ps_sum", bufs=1, space="PSUM"))

    # Constants: tri[k, q] = 1.0 where q >= k (allowed), 0.0 where q < k.
    trif = singles.tile([128, 128], F32)
    make_upper_triangular(nc, trif[:], val=1.0, diag=True)
    tri = singles.tile([128, 128], BF16)
    nc.scalar.copy(out=tri[:], in_=trif[:])
    onesc = singles.tile([128, 1], BF16)   # ones column (sum-over-k lhsT)
    nc.vector.memset(onesc[:], 1.0)

    # One sum bank for the whole program; rows 0/32 and 64/96 alternate by
    # global chunk parity (subtile deps keep the parities independent).
    sum_ps = ps_sum.tile([128, 512], F32)

    kv_tiles = {}

    def ensure_pair(pair):
        if pair in kv_tiles or pair >= PAIRS_PER_CORE:
            return
        kt_sb = kv_pool.tile([D, S], BF16, tag="kt", name=f"kt_{pair}")
        nc.sync.dma_start(out=kt_sb[:], in_=kt[pair])
        v_sb = kv_pool.tile([128, NKT * D], BF16, tag="v", name=f"v_{pair}")
        nc.sync.dma_start(out=v_sb[:], in_=v[pair])
        kv_tiles[pair] = (kt_sb, v_sb)

    q_tiles = {}

    def ensure_head(head):
        if head in q_tiles or head >= HEADS_PER_CORE:
            return
        q_sb = q_pool.tile([D, S], BF16, tag="q", name=f"q_{head}")
        nc.sync.dma_start(out=q_sb[:], in_=qt[head])
        q_tiles[head] = q_sb

    # Flat schedule: (head, chunk, kti)
    sched = []
    for head in range(HEADS_PER_CORE):
        for c in range(NCH):
            for kti in range(8 * c + 8):
                sched.append((head, c, kti))

    # Per-(head,chunk) live state filled in while emitting
    oacc = {}      # (head, c) -> psum accumulator tile
    saccs = {}     # (head, c) -> (tile, base_off)
    osbs = {}      # (head, c) -> sbuf evacuation tile
    stages = {}    # sched index -> staging tile

    # Chunk-tail normalization pipeline, advanced one stage per iteration
    pending = []

    def advance_norm(drain=False):
        for ent in list(pending):
            head, c, st = ent["head"], ent["c"], ent["stage"]
            if st == 0:
                # pull the two sum rows out of PSUM (single partition,
                # 1024 wide) and DMA-reshape them to [128, 8]
                rb = ent["row_base"]
                srow = srow_pool.tile([1, CH], F32, tag="srow",
                                      name=f"srow_{head}_{c}")
                nc.vector.tensor_copy(srow[0:1, 0:512], sum_ps[rb:rb + 1, :])
                nc.vector.tensor_copy(
                    srow[0:1, 512:CH], sum_ps[rb + 32:rb + 33, :])
                srec = srec_pool.tile([128, NCH * 4], F32, tag="srec",
                                      name=f"srec_{head}_{c}")
                nc.sync.dma_start(out=srec[:], in_=srow[:])
                ent["srec"] = srec
            elif st == 1:
                srec2 = srec_pool.tile([128, NCH * 4], BF16, tag="srec2",
                                       name=f"srec2_{head}_{c}")
                with nc.allow_low_precision(reason="1/sum broadcast in bf16"):
                    nc.vector.reciprocal(out=srec2[:], in_=ent["srec"][:])
                nc.sync.dma_start(out=recd[head, c], in_=srec2[:])
            elif st == 2:
                bcs = bcs_pool.tile([128, CH], BF16, tag="bcs", name=f"bcs_{head}_{c}")
                nc.sync.dma_start(
                    out=bcs[:], in_=recd[head, c].partition_broadcast(128))
                ent["bcs"] = bcs
            elif st == 3:
                osb2 = osb_pool.tile([128, CH], BF16, tag="osb2",
                                     name=f"osb2_{head}_{c}")
                nc.vector.tensor_mul(osb2[:], ent["osb"][:], ent["bcs"][:])
                nc.sync.dma_start(
                    out=ot[head][:, CH * c:CH * (c + 1)], in_=osb2[:])
                pending.remove(ent)
            ent["stage"] = st + 1

    def emit_qk(i):
        head, c, kti = sched[i]
        if c == 0 and kti == 0:
            pair = head // G
            ensure_pair(pair + 1)
            ensure_head(head + 1)
        kt_sb, _ = kv_tiles[head // G]
        q_sb = q_tiles[head]
        off = max(0, 128 * kti - CH * c)
        stage = st_pool.tile([128, CH], F32, tag="stage", name=f"st_{i}")
        for (s0, s1) in _segs(off):
            nc.tensor.matmul(
                out=stage[:, s0:s1],
                lhsT=kt_sb[:, 128 * kti:128 * (kti + 1)],
                rhs=q_sb[:, CH * c + s0:CH * c + s1],
                start=True, stop=True,
            )
        stages[i] = stage

    def emit_rest(i):
        head, c, kti = sched[i]
        _, v_sb = kv_tiles[head // G]
        off = max(0, 128 * kti - CH * c)
        last = kti == 8 * c + 7
        row_base = 64 * ((head * NCH + c) % 2)
        stage = stages.pop(i)

        # exp into SBUF (bf16); one wide ACTIVATE per k-tile
        p_kt = pt_pool.tile([128, CH], BF16, tag="pt", name=f"pt_{i}")
        nc.scalar.activation(
            p_kt[:, off:CH], stage[:, off:CH],
            mybir.ActivationFunctionType.Exp, scale=SCALE)

        # causal mask on the diagonal 128x128 block
        if 128 * kti >= CH * c:
            nc.vector.tensor_mul(
                p_kt[:, off:off + 128], p_kt[:, off:off + 128], tri[:])

        # out^T += V^T P^T
        if kti == 0:
            oacc[(head, c)] = oa_pool.tile(
                [128, CH], F32, tag="oacc", name=f"oa_{head}_{c}")
        # causality means cols [0,512) take their final AV write at
        # kti==8c+3; close that bank early so its evacuation overlaps the
        # remaining k-tiles instead of stalling the next chunk's matmuls
        oa = oacc[(head, c)]
        for (s0, s1) in _segs(off):
            bank_last = (kti == 8 * c + 3) if s1 <= 512 else last
            nc.tensor.matmul(
                out=oa[:, s0:s1],
                lhsT=v_sb[:, D * kti:D * (kti + 1)],
                rhs=p_kt[:, s0:s1],
                start=(kti == 0), stop=bank_last,
            )
        if kti == 8 * c + 3:
            osb = osb_pool.tile([128, CH], BF16, tag="osb",
                                name=f"osb_{head}_{c}")
            osbs[(head, c)] = osb
            nc.vector.tensor_copy(osb[:, 0:512], oa[:, 0:512])

        # denominators: a PE/DVE split keeping the PE (the bottleneck
        # engine) light: k-tiles {0,4,8,12} go to the PE as ones-column
        # matmuls, the rest accumulate element-wise on the DVE
        if kti % 4 == 0:
            for (s0, s1) in _segs(off):
                r, c0, c1 = _sum_rc(row_base, s0, s1)
                nc.tensor.matmul(
                    out=sum_ps[r:r + 1, c0:c1],
                    lhsT=onesc[:],
                    rhs=p_kt[:, s0:s1],
                    start=(kti == 0), stop=False,
                    tile_position=(0, r),
                )
        else:
            key = (head, c)
            if key not in saccs:
                sacc = sacc_pool.tile([128, CH], BF16, tag="sacc", name=f"sacc_{head}_{c}")
                nc.vector.tensor_copy(sacc[:, off:CH], p_kt[:, off:CH])
                saccs[key] = (sacc, off)
            else:
                sacc, _ = saccs[key]
                nc.vector.tensor_add(
                    sacc[:, off:CH], sacc[:, off:CH], p_kt[:, off:CH])

        if last:
            # fold the DVE partial sums into the PSUM sum rows
            sacc, base = saccs.pop((head, c))
            for (s0, s1) in _segs(base):
                r, c0, c1 = _sum_rc(row_base, s0, s1)
                nc.tensor.matmul(
                    out=sum_ps[r:r + 1, c0:c1],
                    lhsT=onesc[:],
                    rhs=sacc[:, s0:s1],
                    start=False, stop=True,
                    tile_position=(0, r),
                )
            # evacuate the hi half of the accumulator (lo went at 8c+3)
            oa = oacc.pop((head, c))
            osb = osbs.pop((head, c))
            nc.vector.tensor_copy(osb[:, 512:CH], oa[:, 512:CH])
            pending.append(dict(head=head, c=c, stage=0, osb=osb,
                                row_base=row_base))

    ensure_pair(0)
    ensure_head(0)
    emit_qk(0)
    for i in range(len(sched)):
        if i + 1 < len(sched):
            emit_qk(i + 1)
        emit_rest(i)
        advance_norm()
    while pending:
        advance_norm(drain=True)


_CACHED_NC = None


def build_program():
    global _CACHED_NC
    if _CACHED_NC is not None:
        return _CACHED_NC
    nc = bacc.Bacc("TRN2", target_bir_lowering=False, debug=False,
                   num_devices=NCORES)
    qt = nc.dram_tensor("qt", [HEADS_PER_CORE, D, S], BF16,
                        kind="ExternalInput").ap()
    kt = nc.dram_tensor("kt", [PAIRS_PER_CORE, D, S], BF16,
                        kind="ExternalInput").ap()
    v = nc.dram_tensor("v", [PAIRS_PER_CORE, 128, NKT * D], BF16,
                       kind="ExternalInput").ap()
    recd = nc.dram_tensor("recd", [HEADS_PER_CORE, NCH, CH], BF16,
                          kind="Internal").ap()
    ot = nc.dram_tensor("ot", [HEADS_PER_CORE, D, S], BF16,
                        kind="ExternalOutput").ap()
    with tile.TileContext(nc) as tc:
        emit_core_program(tc, qt, kt, v, recd, ot)
    nc.compile()
    _CACHED_NC = nc
    return nc


def shard_inputs(query, key, value):
    """Full inputs -> list of 8 per-core in_maps (host relayout + bf16)."""
    query = np.asarray(query, dtype=np.float32)
    key = np.asarray(key, dtype=np.float32)
    value = np.asarray(value, dtype=np.float32)

    # Q: [S,B,HQ,D] -> [B*HKV, G, D, S]
    qtall = np.ascontiguousarray(
        query.reshape(S, B, HKV, G, D).transpose(1, 2, 3, 4, 0)
    ).reshape(NPAIRS, G, D, S).astype(NP_BF16)
    # K: [S,B,HKV,D] -> [B*HKV, D, S]
    ktall = np.ascontiguousarray(
        key.transpose(1, 2, 3, 0)).reshape(NPAIRS, D, S).astype(NP_BF16)
    # V: [S,B,HKV,D] -> [B*HKV, k_local=128, NKT*D]
    vall = np.ascontiguousarray(
        value.reshape(NKT, 128, B, HKV, D).transpose(2, 3, 1, 0, 4)
    ).reshape(NPAIRS, 128, NKT * D).astype(NP_BF16)

    in_maps = []
    for c in range(NCORES):
        p0 = PAIRS_PER_CORE * c
        p1 = p0 + PAIRS_PER_CORE
        in_maps.append({
            "qt": np.ascontiguousarray(qtall[p0:p1].reshape(HEADS_PER_CORE, D, S)),
            "kt": np.ascontiguousarray(ktall[p0:p1]),
            "v": np.ascontiguousarray(vall[p0:p1]),
        })
    return in_maps


def unshard_output(results):
    """8 per-core {'ot': [8, D, S]} -> full [S, B, HQ, D]."""
    ot = np.stack([np.asarray(r["ot"], dtype=np.float32) for r in results])
    ot = ot.reshape(B, HKV, G, D, S)                   # pairs major -> b, hkv
    out = np.ascontiguousarray(ot.transpose(4, 0, 1, 2, 3))  # [S,B,HKV,G,D]
    return out.reshape(S, B, HQ, D)


def kernel(query, key, value, _trace=False, _return_bkr=False):
    nc = build_program()
    in_maps = shard_inputs(query, key, value)
    bkr = bass_utils.run_bass_kernel_spmd(
        nc, in_maps, core_ids=list(range(NCORES)), trace=_trace)
    out = unshard_output(bkr.results)
    if _return_bkr:
        return out, bkr
    return out


if __name__ == "__main__":
    q = np.random.randn(S, B, HQ, D).astype(np.float32)
    k = np.random.randn(S, B, HKV, D).astype(np.float32)
    vv = np.random.randn(S, B, HKV, D).astype(np.float32)
    o = kernel(q, k, vv)
    print("out", o.shape, o.dtype, float(np.abs(o).max()))


# revision 22
# speedup vs baseline: 1.3562x; 1.0124x over previous
"""Causal GQA attention (S=2048, B=2, HQ=32, HKV=8, D=128) on 8 trn2 cores.

Sharding: the 16 (batch, kv-head) pairs are split 2 per core (data+head
parallel). Each pair carries group=4 query heads -> 8 attention heads/core.

Per head the device kernel walks two 1024-wide q-chunks; for each chunk it
streams the causal k-tiles (128 wide): S^T = (Q K^T)^T lands in a 2-bank
PSUM staging tile (k on partitions, q on the free axis), one wide ACTIVATE
exponentiates it into SBUF (P^T, bf16), the 128x128 diagonal block is
masked by a triangular multiply, and V-stationary matmuls accumulate
out^T = V^T P^T into a persistent 2-bank PSUM accumulator. All matmul
operands are bf16 (1 col/cycle on the PE at full clock) and every matmul
is <=512 moving columns so no instruction straddles a PSUM bank.

Softmax denominators: k-tiles with kti%4==0 go straight to the PE as
ones-column matmuls into a shared PSUM sum bank (kti==0 opens the
accumulation with full chunk width); the other k-tiles are element-wise
accumulated on the DVE into a bf16 partial-sum tile, which a single pair
of ones-matmuls folds into the same PSUM rows at chunk end. This keeps
both PE and DVE under the ScalarE exp floor, which is the roofline here
(1 elem/lane/cycle @ 1.2 GHz over ~17.4M causal logits/core).

Chunk tails are software-pipelined: the accumulator is evacuated to SBUF
(bf16) immediately so the next chunk's matmuls can reuse the PSUM bank,
then sums -> SBUF -> DMA-reshape [128,8] -> reciprocal -> DRAM ->
partition-broadcast -> multiply -> store advances one stage per k-tile
iteration, several chunks in flight.

Host side only re-lays-out data (and casts to bf16): Q/K as [d, s], V as
[k_local, ktile*d]; the returned out^T [d, s] is transposed/cast back.
"""

import numpy as np
import ml_dtypes

import concourse.bass as bass
import concourse.mybir as mybir
import concourse.tile as tile
from concourse import bacc, bass_utils
from concourse.masks import make_upper_triangular

S, B, HQ, HKV, D = 2048, 2, 32, 8, 128
G = HQ // HKV                      # 4 query heads per kv head
NCORES = 8
NPAIRS = B * HKV                   # 16 (batch, kv-head) pairs
PAIRS_PER_CORE = NPAIRS // NCORES  # 2
HEADS_PER_CORE = PAIRS_PER_CORE * G  # 8
SCALE = 1.0 / float(np.sqrt(D))
CH = 1024                          # q-chunk width (2 PSUM banks)
NCH = S // CH                      # 2
KT = 128                           # k-tile (partition) width
NKT = S // KT                      # 16

F32 = mybir.dt.float32
BF16 = mybir.dt.bfloat16
NP_BF16 = ml_dtypes.bfloat16


def _segs(off):
    """Split chunk cols [off, CH) into <=512 pieces that don't straddle
    the 512 boundary (one PSUM bank per matmul)."""
    if off < 512:
        return [(off, 512), (512, CH)]
    return [(off, CH)]


def _sum_rc(row_base, s0, s1):
    """Map chunk cols [s0, s1) to (row, col range) in the 512-wide sum
    bank: lo half at row_base, hi half at row_base+32."""
    if s0 < 512:
        return row_base, s0, s1
    return row_base + 32, s0 - 512, s1 - 512


def emit_core_program(tc, qt, kt, v, recd, ot):
    from contextlib import ExitStack

    nc = tc.nc
    with ExitStack() as ctx:
        _emit_core_program(ctx, tc, nc, qt, kt, v, recd, ot)


def _emit_core_program(ctx, tc, nc, qt, kt, v, recd, ot):
    singles = ctx.enter_context(tc.tile_pool(name="singles", bufs=1))
    kv_pool = ctx.enter_context(tc.tile_pool(name="kv", bufs=2))
    q_pool = ctx.enter_context(tc.tile_pool(name="q", bufs=2))
    pt_pool = ctx.enter_context(tc.tile_pool(name="pt", bufs=5))
    sacc_pool = ctx.enter_context(tc.tile_pool(name="sacc", bufs=2))
    osb_pool = ctx.enter_context(tc.tile_pool(name="osb", bufs=3))
    bcs_pool = ctx.enter_context(tc.tile_pool(name="bcs", bufs=3))
    srow_pool = ctx.enter_context(tc.tile_pool(name="srow", bufs=3))
    srec_pool = ctx.enter_context(tc.tile_pool(name="srec", bufs=3))
    st_pool = ctx.enter_context(tc.tile_pool(name="st", bufs=2, space="PSUM"))
    oa_pool = ctx.enter_context(tc.tile_pool(name="oa", bufs=1, space="PSUM"))
    ps_sum = ctx.enter_context(tc.tile_pool(name="ps_sum", bufs=1, space="PSUM"))

    # Constants: tri[k, q] = 1.0 where q >= k (allowed), 0.0 where q < k.
    trif = singles.tile([128, 128], F32)
    make_upper_triangular(nc, trif[:], val=1.0, diag=True)
    tri = singles.tile([128, 128], BF16)
    nc.scalar.copy(out=tri[:], in_=trif[:])
    onesc = singles.tile([128, 1], BF16)   # ones column (sum-over-k lhsT)
    nc.vector.memset(onesc[:], 1.0)

    # One sum bank for the whole program; rows 0/32 and 64/96 alternate by
    # global chunk parity (subtile deps keep the parities independent).
    sum_ps = ps_sum.tile([128, 512], F32)

    kv_tiles = {}

    def ensure_pair(pair):
        if pair in kv_tiles or pair >= PAIRS_PER_CORE:
            return
        kt_sb = kv_pool.tile([D, S], BF16, tag="kt", name=f"kt_{pair}")
        nc.sync.dma_start(out=kt_sb[:], in_=kt[pair])
        v_sb = kv_pool.tile([128, NKT * D], BF16, tag="v", name=f"v_{pair}")
        nc.sync.dma_start(out=v_sb[:], in_=v[pair])
        kv_tiles[pair] = (kt_sb, v_sb)

    q_tiles = {}

    def ensure_head(head):
        if head in q_tiles or head >= HEADS_PER_CORE:
            return
        q_sb = q_pool.tile([D, S], BF16, tag="q", name=f"q_{head}")
        nc.sync.dma_start(out=q_sb[:], in_=qt[head])
        q_tiles[head] = q_sb

    # Flat schedule: (head, chunk, kti)
    sched = []
    for head in range(HEADS_PER_CORE):
        for c in range(NCH):
            for kti in range(8 * c + 8):
                sched.append((head, c, kti))

    # Per-(head,chunk) live state filled in while emitting
    oacc = {}      # (head, c) -> psum accumulator tile
    saccs = {}     # (head, c) -> (tile, base_off)
    osbs = {}      # (head, c) -> sbuf evacuation tile
    stages = {}    # sched index -> staging tile

    # Chunk-tail normalization pipeline, advanced one stage per iteration
    pending = []

    def advance_norm(now=-1, drain=False):
        # Process one stage of one (the oldest eligible) entry per call:
        # spreading the chunk-tail DVE/DMA burst across iterations keeps it
        # from delaying the next chunk's masks in the DVE queue.
        for ent in list(pending):
            if not drain and ent["born"] == now:
                continue
            head, c, st = ent["head"], ent["c"], ent["stage"]
            if st == 0:
                # pull the two sum rows out of PSUM (single partition,
                # 1024 wide) and DMA-reshape them to [128, 8]
                rb = ent["row_base"]
                srow = srow_pool.tile([1, CH], F32, tag="srow",
                                      name=f"srow_{head}_{c}")
                nc.vector.tensor_copy(srow[0:1, 0:512], sum_ps[rb:rb + 1, :])
                nc.vector.tensor_copy(
                    srow[0:1, 512:CH], sum_ps[rb + 32:rb + 33, :])
                srec = srec_pool.tile([128, NCH * 4], F32, tag="srec",
                                      name=f"srec_{head}_{c}")
                nc.sync.dma_start(out=srec[:], in_=srow[:])
                ent["srec"] = srec
            elif st == 1:
                srec2 = srec_pool.tile([128, NCH * 4], BF16, tag="srec2",
                                       name=f"srec2_{head}_{c}")
                with nc.allow_low_precision(reason="1/sum broadcast in bf16"):
                    nc.vector.reciprocal(out=srec2[:], in_=ent["srec"][:])
                nc.sync.dma_start(out=recd[head, c], in_=srec2[:])
            elif st == 2:
                bcs = bcs_pool.tile([128, CH], BF16, tag="bcs", name=f"bcs_{head}_{c}")
                nc.sync.dma_start(
                    out=bcs[:], in_=recd[head, c].partition_broadcast(128))
                ent["bcs"] = bcs
            elif st == 3:
                osb2 = osb_pool.tile([128, CH], BF16, tag="osb2",
                                     name=f"osb2_{head}_{c}")
                nc.vector.tensor_mul(osb2[:], ent["osb"][:], ent["bcs"][:])
                nc.sync.dma_start(
                    out=ot[head][:, CH * c:CH * (c + 1)], in_=osb2[:])
                pending.remove(ent)
            ent["stage"] = st + 1
            break

    def emit_qk(i):
        head, c, kti = sched[i]
        if c == 0 and kti == 0:
            pair = head // G
            ensure_pair(pair + 1)
            ensure_head(head + 1)
        kt_sb, _ = kv_tiles[head // G]
        q_sb = q_tiles[head]
        off = max(0, 128 * kti - CH * c)
        stage = st_pool.tile([128, CH], F32, tag="stage", name=f"st_{i}")
        for (s0, s1) in _segs(off):
            nc.tensor.matmul(
                out=stage[:, s0:s1],
                lhsT=kt_sb[:, 128 * kti:128 * (kti + 1)],
                rhs=q_sb[:, CH * c + s0:CH * c + s1],
                start=True, stop=True,
            )
        stages[i] = stage

    def emit_rest(i):
        head, c, kti = sched[i]
        _, v_sb = kv_tiles[head // G]
        off = max(0, 128 * kti - CH * c)
        last = kti == 8 * c + 7
        row_base = 64 * ((head * NCH + c) % 2)
        stage = stages.pop(i)

        # exp into SBUF (bf16); one wide ACTIVATE per k-tile
        p_kt = pt_pool.tile([128, CH], BF16, tag="pt", name=f"pt_{i}")
        nc.scalar.activation(
            p_kt[:, off:CH], stage[:, off:CH],
            mybir.ActivationFunctionType.Exp, scale=SCALE)

        # causal mask on the diagonal 128x128 block
        if 128 * kti >= CH * c:
            nc.vector.tensor_mul(
                p_kt[:, off:off + 128], p_kt[:, off:off + 128], tri[:])

        # out^T += V^T P^T
        if kti == 0:
            oacc[(head, c)] = oa_pool.tile(
                [128, CH], F32, tag="oacc", name=f"oa_{head}_{c}")
        # causality means cols [0,512) take their final AV write at
        # kti==8c+3; close that bank early so its evacuation overlaps the
        # remaining k-tiles instead of stalling the next chunk's matmuls
        oa = oacc[(head, c)]
        for (s0, s1) in _segs(off):
            bank_last = (kti == 8 * c + 3) if s1 <= 512 else last
            nc.tensor.matmul(
                out=oa[:, s0:s1],
                lhsT=v_sb[:, D * kti:D * (kti + 1)],
                rhs=p_kt[:, s0:s1],
                start=(kti == 0), stop=bank_last,
            )
        if kti == 8 * c + 3:
            osb = osb_pool.tile([128, CH], BF16, tag="osb",
                                name=f"osb_{head}_{c}")
            osbs[(head, c)] = osb
            nc.vector.tensor_copy(osb[:, 0:512], oa[:, 0:512])

        # denominators: a PE/DVE split keeping the PE (the bottleneck
        # engine) light: k-tiles {0,4,8,12} go to the PE as ones-column
        # matmuls, the rest accumulate element-wise on the DVE
        if kti % 4 == 0:
            for (s0, s1) in _segs(off):
                r, c0, c1 = _sum_rc(row_base, s0, s1)
                nc.tensor.matmul(
                    out=sum_ps[r:r + 1, c0:c1],
                    lhsT=onesc[:],
                    rhs=p_kt[:, s0:s1],
                    start=(kti == 0), stop=False,
                    tile_position=(0, r),
                )
        else:
            key = (head, c)
            if key not in saccs:
                sacc = sacc_pool.tile([128, CH], BF16, tag="sacc", name=f"sacc_{head}_{c}")
                nc.vector.tensor_copy(sacc[:, off:CH], p_kt[:, off:CH])
                saccs[key] = (sacc, off)
            else:
                sacc, _ = saccs[key]
                nc.vector.tensor_add(
                    sacc[:, off:CH], sacc[:, off:CH], p_kt[:, off:CH])

        if last:
            # fold the DVE partial sums into the PSUM sum rows
            sacc, base = saccs.pop((head, c))
            for (s0, s1) in _segs(base):
                r, c0, c1 = _sum_rc(row_base, s0, s1)
                nc.tensor.matmul(
                    out=sum_ps[r:r + 1, c0:c1],
                    lhsT=onesc[:],
                    rhs=sacc[:, s0:s1],
                    start=False, stop=True,
                    tile_position=(0, r),
                )
            # evacuate the hi half of the accumulator (lo went at 8c+3)
            oa = oacc.pop((head, c))
            osb = osbs.pop((head, c))
            nc.vector.tensor_copy(osb[:, 512:CH], oa[:, 512:CH])
            pending.append(dict(head=head, c=c, stage=0, osb=osb,
                                row_base=row_base, born=i))

    ensure_pair(0)
    ensure_head(0)
    emit_qk(0)
    for i in range(len(sched)):
        if i + 1 < len(sched):
            emit_qk(i + 1)
        emit_rest(i)
        advance_norm(now=i)
    while pending:
        advance_norm(drain=True)


_CACHED_NC = None


def build_program():
    global _CACHED_NC
    if _CACHED_NC is not None:
        return _CACHED_NC
    nc = bacc.Bacc("TRN2", target_bir_lowering=False, debug=False,
                   num_devices=NCORES)
    qt = nc.dram_tensor("qt", [HEADS_PER_CORE, D, S], BF16,
                        kind="ExternalInput").ap()
    kt = nc.dram_tensor("kt", [PAIRS_PER_CORE, D, S], BF16,
                        kind="ExternalInput").ap()
    v = nc.dram_tensor("v", [PAIRS_PER_CORE, 128, NKT * D], BF16,
                       kind="ExternalInput").ap()
    recd = nc.dram_tensor("recd", [HEADS_PER_CORE, NCH, CH], BF16,
                          kind="Internal").ap()
    ot = nc.dram_tensor("ot", [HEADS_PER_CORE, D, S], BF16,
                        kind="ExternalOutput").ap()
    with tile.TileContext(nc) as tc:
        emit_core_program(tc, qt, kt, v, recd, ot)
    nc.compile()
    _CACHED_NC = nc
    return nc


def shard_inputs(query, key, value):
    """Full inputs -> list of 8 per-core in_maps (host relayout + bf16)."""
    query = np.asarray(query, dtype=np.float32)
    key = np.asarray(key, dtype=np.float32)
    value = np.asarray(value, dtype=np.float32)

    # Q: [S,B,HQ,D] -> [B*HKV, G, D, S]
    qtall = np.ascontiguousarray(
        query.reshape(S, B, HKV, G, D).transpose(1, 2, 3, 4, 0)
    ).reshape(NPAIRS, G, D, S).astype(NP_BF16)
    # K: [S,B,HKV,D] -> [B*HKV, D, S]
    ktall = np.ascontiguousarray(
        key.transpose(1, 2, 3, 0)).reshape(NPAIRS, D, S).astype(NP_BF16)
    # V: [S,B,HKV,D] -> [B*HKV, k_local=128, NKT*D]
    vall = np.ascontiguousarray(
        value.reshape(NKT, 128, B, HKV, D).transpose(2, 3, 1, 0, 4)
    ).reshape(NPAIRS, 128, NKT * D).astype(NP_BF16)

    in_maps = []
    for c in range(NCORES):
        p0 = PAIRS_PER_CORE * c
        p1 = p0 + PAIRS_PER_CORE
        in_maps.append({
            "qt": np.ascontiguousarray(qtall[p0:p1].reshape(HEADS_PER_CORE, D, S)),
            "kt": np.ascontiguousarray(ktall[p0:p1]),
            "v": np.ascontiguousarray(vall[p0:p1]),
        })
    return in_maps


def unshard_output(results):
    """8 per-core {'ot': [8, D, S]} -> full [S, B, HQ, D]."""
    ot = np.stack([np.asarray(r["ot"], dtype=np.float32) for r in results])
    ot = ot.reshape(B, HKV, G, D, S)                   # pairs major -> b, hkv
    out = np.ascontiguousarray(ot.transpose(4, 0, 1, 2, 3))  # [S,B,HKV,G,D]
    return out.reshape(S, B, HQ, D)


def kernel(query, key, value, _trace=False, _return_bkr=False):
    nc = build_program()
    in_maps = shard_inputs(query, key, value)
    bkr = bass_utils.run_bass_kernel_spmd(
        nc, in_maps, core_ids=list(range(NCORES)), trace=_trace)
    out = unshard_output(bkr.results)
    if _return_bkr:
        return out, bkr
    return out


if __name__ == "__main__":
    q = np.random.randn(S, B, HQ, D).astype(np.float32)
    k = np.random.randn(S, B, HKV, D).astype(np.float32)
    vv = np.random.randn(S, B, HKV, D).astype(np.float32)
    o = kernel(q, k, vv)
    print("out", o.shape, o.dtype, float(np.abs(o).max()))
